# revision 17
# baseline (speedup 1.0000x reference)
"""Bidirectional toroidal lattice message passing on 8 Trainium2 cores (v4).

The [N,N] adjacencies are toroidal 3-neighbor shift operators (verified on
host; dense fallback otherwise). The 10-step recurrence

  x_{s+1} = c1 x_s + g (.) Op(x_s)         (Op = the 3-shift stencil)

is reformulated so the state lives IN PSUM and self-accumulates: with
P_s := psum_s * c1^{-s} and ghat := g/c1,

  P_{s+1} = P_s + Op(ghat (.) P_s)

so the per-step critical path is just: matmul-pair (accumulating into a
persistent psum bank) -> one DVE multiply (m_s = ghat (.) P_s, bf16 out).
The step states are recovered from  acc = W0*x0 + sum_j wtilde_j m_j  with
scalar weights: j=1..8 accumulate on device (Act scale-copy + Pool add for
fwd, one DVE scalar_tensor_tensor for rev), the j=0 and j=9 terms are added
on host (m_0 = ghat (.) Op(x0) is host-packed input, m_9 is DMA'd out raw),
which removes one serial V round-trip at the start and the acc drain at the
end. Final combine (f + r + sig*f*r) is host numpy.

The reverse direction is stored point-reflected (theta & phi mirrored), which
turns its (-1) shifts into (+1) shifts: both directions share the same two
bf16 stationaries S=[shift] and M=[I+S], and the two serial chains overlap
(fwd's next matmuls start while rev's DVE mul still runs). Phi wrap is
handled by a 74-wide (64 + 10-step creep) column domain packed on host — no
per-step halo copies. Batch is sharded 2-per-core across 8 cores; no
collectives.
"""

import numpy as np

NT, NP, S = 128, 64, 10
N = NT * NP
B = 16
NCORES = 8
BPC = B // NCORES  # batches per core
HALO = S           # left garbage-creep columns (1 per step)
W = NP + HALO      # 74 phi columns; col c <-> phi = (c - HALO) mod 64

_FWD = [(1, 0), (0, 1), (1, 1)]
_REV = [(-1, 0), (0, -1), (-1, -1)]


def _diag_vals(adj, shifts):
    idx = np.arange(N)
    ti, pi = idx // NP, idx % NP
    return [adj[idx, ((ti + dt) % NT) * NP + (pi + dp) % NP] for dt, dp in shifts]


def _softmax(x):
    e = np.exp(x - x.max())
    return (e / e.sum()).astype(np.float32)


def _structure_ok(adj, vals):
    for v in vals:
        if np.ptp(v) > 1e-6 * max(1.0, abs(float(v.mean()))):
            return False
    total = adj.sum(dtype=np.float64)
    diag = sum(v.sum(dtype=np.float64) for v in vals)
    return abs(total - diag) < 1e-3


def _reference_fallback(entry, fwd_adj, rev_adj, fwd_sw, fwd_decay, rev_sw,
                        rev_decay, iw, angles):
    # generic dense path (host); only used if the adjacency is not the
    # expected toroidal shift structure.
    def prop(adj, decay, sw):
        d = float(np.clip(decay, 0.5, 0.99))
        af = 0.5 + 0.5 * np.cos(np.abs(angles).mean(axis=1))
        x = entry.astype(np.float32)
        w = _softmax(np.asarray(sw, np.float32))
        acc = np.zeros_like(x)
        for s in range(S):
            p = (x @ adj) * af[None, :]
            x = ((0.3 * x + 0.7 * p) * d).astype(np.float32)
            acc += w[s] * x
        return acc
    f = prop(fwd_adj, fwd_decay, fwd_sw)
    r = prop(rev_adj, rev_decay, rev_sw)
    inter = f * r
    sig = 1.0 / (1.0 + np.exp(-float(iw)))
    return (f + r + np.float32(sig) * inter).astype(np.float32), inter.astype(np.float32)


def _acc_weights(w, c1):
    """acc = sum_t w[t-1] x_t = W0*x0 + sum_j wtilde_j * m~_j."""
    W0 = float(sum(w[t - 1] * c1 ** t for t in range(1, S + 1)))
    wt = [float(c1 ** (j + 1) *
                sum(w[t - 1] * c1 ** (t - 1 - j) for t in range(j + 1, S + 1)))
          for j in range(S)]
    return W0, wt


def _build_program(wts):
    """SPMD Bass program (identical on all cores)."""
    import concourse.bacc as bacc
    import concourse.mybir as mybir
    from concourse.tile import TileContext

    fp32 = mybir.dt.float32
    bf16 = mybir.dt.bfloat16
    i32 = mybir.dt.int32
    OP = mybir.AluOpType
    ACT = mybir.ActivationFunctionType

    wtf, wtr = wts

    nc = bacc.Bacc(None, target_bir_lowering=False)

    # packed input: [theta, kind (x0 | m~_0), dir, b, col] bf16
    xm_d = nc.dram_tensor("xm", [NT, 2, 2, BPC, W], bf16, kind="ExternalInput")
    gs_d = nc.dram_tensor("gs", [NT, 2, BPC, W], fp32, kind="ExternalInput")
    # outputs: device acc over j=1..8, and raw m~_9 (both SBUF-layout-matched)
    acc_d = nc.dram_tensor("acc8", [NT, 2, BPC, NP], fp32, kind="ExternalOutput")
    m9_d = nc.dram_tensor("m9", [NT, 2, BPC, NP], bf16, kind="ExternalOutput")

    same_w = all(abs(a - b) <= 1e-12 * max(abs(a), abs(b), 1e-30)
                 for a, b in zip(wtf, wtr))

    with TileContext(nc) as tc:
        with (
            tc.tile_pool(name="sb", bufs=1) as spool,
            tc.tile_pool(name="psum", bufs=1, space="PSUM") as ppool,
        ):
            xm = spool.tile([NT, 2, 2, BPC, W], bf16, tag="xm")
            gs = spool.tile([NT, 2, BPC, W], fp32, tag="gs")
            nc.sync.dma_start(xm[:], xm_d[:])
            nc.scalar.dma_start(gs[:], gs_d[:])

            # stationaries: v[k,i] = (i-k) mod 128 ; S = [v==1], M = [v<2]
            mats = spool.tile([NT, 2, NT], bf16, tag="mats")
            v = spool.tile([NT, NT], i32, tag="v")
            nc.gpsimd.iota(v[:], pattern=[[1, NT]], base=NT,
                           channel_multiplier=-1)
            nc.vector.tensor_scalar(v[:], v[:], scalar1=NT - 1, scalar2=None,
                                    op0=OP.bitwise_and)
            nc.vector.tensor_scalar(mats[:, 0], v[:], scalar1=1, scalar2=None,
                                    op0=OP.is_equal)
            nc.vector.tensor_scalar(mats[:, 1], v[:], scalar1=2, scalar2=None,
                                    op0=OP.is_lt)
            Smat, Mmat = mats[:, 0], mats[:, 1]

            # persistent psum accumulators: one 2-bank tile (2048B per dir)
            # so wide 2-dir views are a single strided AP
            pt = ppool.tile([NT, 2, 512], fp32, tag="P")
            P = [pt[:, d, 0:BPC * W].rearrange("t (b w) -> t b w", b=BPC)
                 for d in (0, 1)]

            out_t = spool.tile([NT, 2, BPC, NP], fp32, tag="out_t")
            m9 = spool.tile([NT, 2, BPC, NP], bf16, tag="m9")

            acc = None   # wide-acc fast path (same weights both dirs)
            accs = [None, None]
            # step j: matmul pair accumulates P_j from m~_{j-1}
            # (j=0: x0, j=1: host-packed m~_0, else device m~ tiles)
            mprev = [xm[:, 0, 0], xm[:, 0, 1]]
            for j in range(S):
                lo = j + 1
                for d in (0, 1):  # per-dir grouping: fwd chain unblocks early
                    mv = mprev[d]
                    nc.tensor.matmul(P[d][:, :, lo:W], Smat, mv[:, :, lo:W],
                                     start=(j == 0), stop=False,
                                     skip_group_check=True)
                    nc.tensor.matmul(P[d][:, :, lo:W], Mmat,
                                     mv[:, :, lo - 1:W - 1],
                                     start=False, stop=True,
                                     skip_group_check=True)
                if j == 0:
                    mprev = [xm[:, 1, 0], xm[:, 1, 1]]
                    continue

                # chain op: m~_j = ghat (.) P_j  (bf16 out); one mul per dir
                # keeps the two serial chains decoupled
                if j == S - 1:
                    for d in (0, 1):
                        nc.vector.tensor_mul(m9[:, d], P[d][:, :, HALO:W],
                                             gs[:, d, :, HALO:W])
                    break
                m = spool.tile([NT, 2, BPC, W], bf16, tag="m", bufs=3,
                               name=f"m_{j}")
                for d in (0, 1):
                    nc.vector.tensor_mul(m[:, d, :, lo:W], P[d][:, :, lo:W],
                                         gs[:, d, :, lo:W])

                # off-chain acc over j=1..8 (center cols, both dirs at once
                # when the step weights coincide): Act copy-scale + Pool add
                mc = m[:, :, :, HALO:W]
                if same_w:
                    mp = spool.tile([NT, 2, BPC, NP], fp32, tag="mp", bufs=2,
                                    name=f"mp_{j}")
                    nc.scalar.activation(mp[:], mc, ACT.Copy,
                                         bias=0.0, scale=float(wtf[j]))
                    if j == 1:
                        acc = mp
                    else:
                        na = out_t if j == S - 2 else spool.tile(
                            [NT, 2, BPC, NP], fp32, tag="acc", bufs=2,
                            name=f"acc_{j}")
                        nc.gpsimd.tensor_add(na[:], acc[:], mp[:])
                        acc = na
                else:
                    wt = (wtf, wtr)
                    for d in (0, 1):
                        mp = spool.tile([NT, BPC, NP], fp32, tag=f"mp{d}",
                                        bufs=2, name=f"mp{d}_{j}")
                        nc.scalar.activation(mp[:], mc[:, d], ACT.Copy,
                                             bias=0.0, scale=float(wt[d][j]))
                        if j == 1:
                            accs[d] = mp
                        else:
                            na = out_t[:, d] if j == S - 2 else spool.tile(
                                [NT, BPC, NP], fp32, tag=f"acc{d}", bufs=2,
                                name=f"acc{d}_{j}")
                            nc.gpsimd.tensor_add(na[:], accs[d][:], mp[:])
                            accs[d] = na
                mprev = [m[:, 0], m[:, 1]]

            nc.sync.dma_start(acc_d[:], out_t[:])
            nc.scalar.dma_start(m9_d[:], m9[:])

    nc.finalize()
    return nc


def _host_prep(inputs):
    import ml_dtypes

    entry = np.ascontiguousarray(np.asarray(inputs["entry_probs"], np.float32))
    fwd_adj = np.asarray(inputs["forward_adj"], np.float32)
    rev_adj = np.asarray(inputs["reverse_adj"], np.float32)
    angles = np.asarray(inputs["bounce_angles"], np.float32)

    vf = _diag_vals(fwd_adj, _FWD)
    vr = _diag_vals(rev_adj, _REV)
    ok = _structure_ok(fwd_adj, vf) and _structure_ok(rev_adj, vr)

    df = float(np.clip(float(np.asarray(inputs["forward_decay"])), 0.5, 0.99))
    dr = float(np.clip(float(np.asarray(inputs["reverse_decay"])), 0.5, 0.99))
    wf = _softmax(np.asarray(inputs["forward_step_weights"], np.float32))
    wr = _softmax(np.asarray(inputs["reverse_step_weights"], np.float32))
    sig = float(1.0 / (1.0 + np.exp(-float(np.asarray(inputs["interaction_weight"])))))

    vbf = [float(v.mean()) for v in vf]   # [v10, v01, v11]
    vbr = [float(v.mean()) for v in vr]
    # 0/1 shift matrices require one shared constant per direction
    for vs in (vbf, vbr):
        if abs(vs[0] - vs[1]) > 1e-6 * abs(vs[0]) or \
           abs(vs[0] - vs[2]) > 1e-6 * abs(vs[0]):
            ok = False

    c1f, c1r = 0.3 * df, 0.3 * dr
    af2 = (0.5 + 0.5 * np.cos(np.abs(angles).mean(axis=1))) \
        .astype(np.float32).reshape(NT, NP)
    gf = (0.7 * df * vbf[0]) * af2            # [128, 64]
    gr = (0.7 * dr * vbr[0]) * af2

    invt = (-np.arange(NT)) % NT
    invp = (-np.arange(NP)) % NP
    grm = gr[invt][:, invp]                   # mirrored rev gain field

    colphi = (np.arange(W) - HALO) % NP       # col -> phi
    ghat = np.empty((NT, 2, BPC, W), np.float32)
    ghat[:, 0] = (gf / c1f)[:, None, colphi]
    ghat[:, 1] = (grm / c1r)[:, None, colphi]

    W0f, wtf = _acc_weights(wf, c1f)
    W0r, wtr = _acc_weights(wr, c1r)

    # per-core packs (bf16): x0 (fwd plain / rev point-mirrored) and the
    # host-computed first gated update m~_0 = ghat (.) Op(x0)
    e3 = entry.reshape(B, NT, NP)
    em = e3[:, invt][:, :, invp]
    xm_list = []
    m0_list = []
    for c in range(NCORES):
        x0 = np.empty((NT, 2, BPC, W), np.float32)
        x0[:, 0] = e3[c * BPC:(c + 1) * BPC][:, :, colphi].transpose(1, 0, 2)
        x0[:, 1] = em[c * BPC:(c + 1) * BPC][:, :, colphi].transpose(1, 0, 2)
        x0 = x0.astype(ml_dtypes.bfloat16).astype(np.float32)
        x0m = np.roll(x0, 1, axis=0)          # theta-1 (S wraps partitions)
        p0 = np.zeros_like(x0)
        p0[..., 1:] = x0m[..., 1:] + x0[..., :-1] + x0m[..., :-1]
        m0 = (ghat.transpose(0, 1, 2, 3) * p0).astype(ml_dtypes.bfloat16)
        m0[..., 0] = 0
        xm = np.stack([x0.astype(ml_dtypes.bfloat16), m0], axis=1)
        xm_list.append(np.ascontiguousarray(xm))  # [NT, 2, 2, BPC, W]
        m0_list.append(m0.astype(np.float32))
    meta = dict(
        ok=ok, sig=sig,
        W0s=(W0f, W0r), wts=(tuple(wtf), tuple(wtr)),
        gs=np.ascontiguousarray(ghat), xm_list=xm_list, m0_list=m0_list,
        invt=invt, invp=invp, e3=e3, em=em,
    )
    return meta


_PROGRAM_CACHE = {}
LAST_RESULT = None


def kernel(**inputs):
    meta = _host_prep(inputs)
    if not meta["ok"]:
        return _reference_fallback(
            np.asarray(inputs["entry_probs"], np.float32),
            np.asarray(inputs["forward_adj"], np.float32),
            np.asarray(inputs["reverse_adj"], np.float32),
            inputs["forward_step_weights"], inputs["forward_decay"],
            inputs["reverse_step_weights"], inputs["reverse_decay"],
            inputs["interaction_weight"], np.asarray(inputs["bounce_angles"], np.float32))

    # If tracing is requested via BASS_TRACE but the image's antenv lacks
    # axon_hooks, provide the hook so run_bass_kernel_spmd doesn't crash.
    import os as _os
    if _os.environ.get("BASS_TRACE"):
        try:
            import antenv.axon_hooks  # noqa: F401
        except ImportError:
            try:
                import sys as _sys
                import types as _types
                import trn_agent_boot.trn_boot as _tb
                _hook = _tb._ntff_profile_via_ctypes("/opt/axon/libaxon_pjrt.so")
                _mod = _types.ModuleType("antenv.axon_hooks")
                _mod.get_axon_ntff_profile_hook = lambda: _hook
                _mod.set_axon_ntff_profile_hook = lambda h: None
                _sys.modules["antenv.axon_hooks"] = _mod
            except Exception:
                _os.environ.pop("BASS_TRACE", None)

    from concourse import bass_utils

    key = meta["wts"]
    if key not in _PROGRAM_CACHE:
        _PROGRAM_CACHE[key] = _build_program(meta["wts"])
    nc = _PROGRAM_CACHE[key]

    in_maps = [{"xm": meta["xm_list"][c], "gs": meta["gs"]}
               for c in range(NCORES)]
    res = bass_utils.run_bass_kernel_spmd(nc, in_maps, core_ids=list(range(NCORES)))
    global LAST_RESULT
    LAST_RESULT = res

    (W0f, W0r), (wtf, wtr) = meta["W0s"], meta["wts"]

    def gather(name, dtype):
        # [C, NT, 2, BPC, NP] -> [2, B, N]
        a = np.stack([np.asarray(r[name]).astype(dtype) for r in res.results])
        return a.transpose(2, 0, 3, 1, 4).reshape(2, B, N)

    acc8 = gather("acc8", np.float32)
    m9 = gather("m9", np.float32)
    m0 = np.stack([m[:, :, :, HALO:W] for m in meta["m0_list"]]) \
        .transpose(2, 0, 3, 1, 4).reshape(2, B, N)

    f = (W0f * meta["e3"].reshape(B, N) + wtf[0] * m0[0] + acc8[0]
         + wtf[S - 1] * m9[0])
    rm = (W0r * meta["em"].reshape(B, N) + wtr[0] * m0[1] + acc8[1]
          + wtr[S - 1] * m9[1])
    rm3 = rm.reshape(B, NT, NP)
    r = rm3[:, meta["invt"]][:, :, meta["invp"]].reshape(B, N)
    f = f.astype(np.float32)
    r = r.astype(np.float32)
    inter = (f * r).astype(np.float32)
    comb = (f + r + np.float32(meta["sig"]) * inter).astype(np.float32)
    return comb, inter


# revision 18
# speedup vs baseline: 1.1586x; 1.1586x over previous
"""Bidirectional toroidal lattice message passing on 8 Trainium2 cores (v4).

The [N,N] adjacencies are toroidal 3-neighbor shift operators (verified on
host; dense fallback otherwise). The 10-step recurrence

  x_{s+1} = c1 x_s + g (.) Op(x_s)         (Op = the 3-shift stencil)

is reformulated so the state lives IN PSUM and self-accumulates: with
P_s := psum_s * c1^{-s} and ghat := g/c1,

  P_{s+1} = P_s + Op(ghat (.) P_s)

so the per-step critical path is just: matmul-pair (accumulating into a
persistent psum bank) -> one DVE multiply (m_s = ghat (.) P_s, bf16 out).
The step states are recovered from  acc = W0*x0 + sum_j wtilde_j m_j  with
scalar weights: j=1..8 accumulate on device (Act scale-copy + Pool add for
fwd, one DVE scalar_tensor_tensor for rev), the j=0 and j=9 terms are added
on host (m_0 = ghat (.) Op(x0) is host-packed input, m_9 is DMA'd out raw),
which removes one serial V round-trip at the start and the acc drain at the
end. Final combine (f + r + sig*f*r) is host numpy.

The reverse direction is stored point-reflected (theta & phi mirrored), which
turns its (-1) shifts into (+1) shifts: both directions share the same two
bf16 stationaries S=[shift] and M=[I+S], and the two serial chains overlap
(fwd's next matmuls start while rev's DVE mul still runs). Phi wrap is
handled by a 74-wide (64 + 10-step creep) column domain packed on host — no
per-step halo copies. Batch is sharded 2-per-core across 8 cores; no
collectives.
"""

import numpy as np

NT, NP, S = 128, 64, 10
N = NT * NP
B = 16
NCORES = 8
BPC = B // NCORES  # batches per core
HALO = S           # left garbage-creep columns (1 per step)
W = NP + HALO      # 74 phi columns; col c <-> phi = (c - HALO) mod 64

_FWD = [(1, 0), (0, 1), (1, 1)]
_REV = [(-1, 0), (0, -1), (-1, -1)]


def _diag_vals(adj, shifts):
    idx = np.arange(N)
    ti, pi = idx // NP, idx % NP
    return [adj[idx, ((ti + dt) % NT) * NP + (pi + dp) % NP] for dt, dp in shifts]


def _softmax(x):
    e = np.exp(x - x.max())
    return (e / e.sum()).astype(np.float32)


def _structure_ok(adj, vals):
    for v in vals:
        if np.ptp(v) > 1e-6 * max(1.0, abs(float(v.mean()))):
            return False
    total = adj.sum(dtype=np.float64)
    diag = sum(v.sum(dtype=np.float64) for v in vals)
    return abs(total - diag) < 1e-3


def _reference_fallback(entry, fwd_adj, rev_adj, fwd_sw, fwd_decay, rev_sw,
                        rev_decay, iw, angles):
    # generic dense path (host); only used if the adjacency is not the
    # expected toroidal shift structure.
    def prop(adj, decay, sw):
        d = float(np.clip(decay, 0.5, 0.99))
        af = 0.5 + 0.5 * np.cos(np.abs(angles).mean(axis=1))
        x = entry.astype(np.float32)
        w = _softmax(np.asarray(sw, np.float32))
        acc = np.zeros_like(x)
        for s in range(S):
            p = (x @ adj) * af[None, :]
            x = ((0.3 * x + 0.7 * p) * d).astype(np.float32)
            acc += w[s] * x
        return acc
    f = prop(fwd_adj, fwd_decay, fwd_sw)
    r = prop(rev_adj, rev_decay, rev_sw)
    inter = f * r
    sig = 1.0 / (1.0 + np.exp(-float(iw)))
    return (f + r + np.float32(sig) * inter).astype(np.float32), inter.astype(np.float32)


def _acc_weights(w, c1):
    """acc = sum_t w[t-1] x_t = W0*x0 + sum_j wtilde_j * m~_j."""
    W0 = float(sum(w[t - 1] * c1 ** t for t in range(1, S + 1)))
    wt = [float(c1 ** (j + 1) *
                sum(w[t - 1] * c1 ** (t - 1 - j) for t in range(j + 1, S + 1)))
          for j in range(S)]
    return W0, wt


def _build_program(wts):
    """SPMD Bass program (identical on all cores)."""
    import concourse.bacc as bacc
    import concourse.mybir as mybir
    from concourse.tile import TileContext

    fp32 = mybir.dt.float32
    bf16 = mybir.dt.bfloat16
    i32 = mybir.dt.int32
    OP = mybir.AluOpType
    ACT = mybir.ActivationFunctionType

    wtf, wtr = wts

    nc = bacc.Bacc(None, target_bir_lowering=False)

    # packed input: [theta, kind (x0 | m~_0), dir, b, col] bf16
    xm_d = nc.dram_tensor("xm", [NT, 2, 2, BPC, W], bf16, kind="ExternalInput")
    gs_d = nc.dram_tensor("gs", [NT, 2, BPC, W], fp32, kind="ExternalInput")
    # outputs: device acc over j=1..8, and raw m~_9 (both SBUF-layout-matched)
    acc_d = nc.dram_tensor("acc8", [NT, 2, BPC, NP], fp32, kind="ExternalOutput")
    m9_d = nc.dram_tensor("m9", [NT, 2, BPC, NP], bf16, kind="ExternalOutput")

    same_w = all(abs(a - b) <= 1e-12 * max(abs(a), abs(b), 1e-30)
                 for a, b in zip(wtf, wtr))

    with TileContext(nc) as tc:
        with (
            tc.tile_pool(name="sb", bufs=1) as spool,
            tc.tile_pool(name="psum", bufs=1, space="PSUM") as ppool,
        ):
            xm = spool.tile([NT, 2, 2, BPC, W], bf16, tag="xm")
            gs = spool.tile([NT, 2, BPC, W], fp32, tag="gs")
            nc.sync.dma_start(xm[:], xm_d[:])
            nc.scalar.dma_start(gs[:], gs_d[:])

            # stationaries: v[k,i] = (i-k) mod 128 ; S = [v==1], M = [v<2]
            mats = spool.tile([NT, 2, NT], bf16, tag="mats")
            v = spool.tile([NT, NT], i32, tag="v")
            nc.gpsimd.iota(v[:], pattern=[[1, NT]], base=NT,
                           channel_multiplier=-1)
            nc.vector.tensor_scalar(v[:], v[:], scalar1=NT - 1, scalar2=None,
                                    op0=OP.bitwise_and)
            nc.vector.tensor_scalar(mats[:, 0], v[:], scalar1=1, scalar2=None,
                                    op0=OP.is_equal)
            nc.vector.tensor_scalar(mats[:, 1], v[:], scalar1=2, scalar2=None,
                                    op0=OP.is_lt)
            Smat, Mmat = mats[:, 0], mats[:, 1]

            # persistent psum accumulators, one bank per direction
            Pf = ppool.tile([NT, BPC, W], fp32, tag="Pf")
            Pr = ppool.tile([NT, BPC, W], fp32, tag="Pr")
            P = [Pf, Pr]

            out_t = spool.tile([NT, 2, BPC, NP], fp32, tag="out_t")
            m9 = spool.tile([NT, 2, BPC, NP], bf16, tag="m9")

            acc = None   # wide-acc fast path (same weights both dirs)
            accs = [None, None]
            # step j: matmul pair accumulates P_j from m~_{j-1}
            # (j=0: x0, j=1: host-packed m~_0, else device m~ tiles)
            mprev = [xm[:, 0, 0], xm[:, 0, 1]]
            for j in range(S):
                lo = j + 1
                for d in (0, 1):  # per-dir grouping: fwd chain unblocks early
                    mv = mprev[d]
                    nc.tensor.matmul(P[d][:, :, lo:W], Smat, mv[:, :, lo:W],
                                     start=(j == 0), stop=False,
                                     skip_group_check=True)
                    nc.tensor.matmul(P[d][:, :, lo:W], Mmat,
                                     mv[:, :, lo - 1:W - 1],
                                     start=False, stop=True,
                                     skip_group_check=True)
                if j == 0:
                    mprev = [xm[:, 1, 0], xm[:, 1, 1]]
                    continue

                # chain op: m~_j = ghat (.) P_j  (bf16 out); one mul per dir
                # keeps the two serial chains decoupled
                if j == S - 1:
                    for d in (0, 1):
                        nc.vector.tensor_mul(m9[:, d], P[d][:, :, HALO:W],
                                             gs[:, d, :, HALO:W])
                    break
                m = spool.tile([NT, 2, BPC, W], bf16, tag="m", bufs=3,
                               name=f"m_{j}")
                for d in (0, 1):
                    nc.vector.tensor_mul(m[:, d, :, lo:W], P[d][:, :, lo:W],
                                         gs[:, d, :, lo:W])

                # off-chain acc over j=1..8 (center cols, both dirs at once
                # when the step weights coincide): Act copy-scale + Pool add
                mc = m[:, :, :, HALO:W]
                if same_w:
                    mp = spool.tile([NT, 2, BPC, NP], fp32, tag="mp", bufs=2,
                                    name=f"mp_{j}")
                    nc.scalar.activation(mp[:], mc, ACT.Copy,
                                         bias=0.0, scale=float(wtf[j]))
                    if j == 1:
                        acc = mp
                    else:
                        na = out_t if j == S - 2 else spool.tile(
                            [NT, 2, BPC, NP], fp32, tag="acc", bufs=2,
                            name=f"acc_{j}")
                        nc.gpsimd.tensor_add(na[:], acc[:], mp[:])
                        acc = na
                else:
                    wt = (wtf, wtr)
                    for d in (0, 1):
                        mp = spool.tile([NT, BPC, NP], fp32, tag=f"mp{d}",
                                        bufs=2, name=f"mp{d}_{j}")
                        nc.scalar.activation(mp[:], mc[:, d], ACT.Copy,
                                             bias=0.0, scale=float(wt[d][j]))
                        if j == 1:
                            accs[d] = mp
                        else:
                            na = out_t[:, d] if j == S - 2 else spool.tile(
                                [NT, BPC, NP], fp32, tag=f"acc{d}", bufs=2,
                                name=f"acc{d}_{j}")
                            nc.gpsimd.tensor_add(na[:], accs[d][:], mp[:])
                            accs[d] = na
                mprev = [m[:, 0], m[:, 1]]

            nc.sync.dma_start(acc_d[:], out_t[:])
            nc.scalar.dma_start(m9_d[:], m9[:])

    nc.finalize()
    return nc


def _host_prep(inputs):
    import ml_dtypes

    entry = np.ascontiguousarray(np.asarray(inputs["entry_probs"], np.float32))
    fwd_adj = np.asarray(inputs["forward_adj"], np.float32)
    rev_adj = np.asarray(inputs["reverse_adj"], np.float32)
    angles = np.asarray(inputs["bounce_angles"], np.float32)

    vf = _diag_vals(fwd_adj, _FWD)
    vr = _diag_vals(rev_adj, _REV)
    ok = _structure_ok(fwd_adj, vf) and _structure_ok(rev_adj, vr)

    df = float(np.clip(float(np.asarray(inputs["forward_decay"])), 0.5, 0.99))
    dr = float(np.clip(float(np.asarray(inputs["reverse_decay"])), 0.5, 0.99))
    wf = _softmax(np.asarray(inputs["forward_step_weights"], np.float32))
    wr = _softmax(np.asarray(inputs["reverse_step_weights"], np.float32))
    sig = float(1.0 / (1.0 + np.exp(-float(np.asarray(inputs["interaction_weight"])))))

    vbf = [float(v.mean()) for v in vf]   # [v10, v01, v11]
    vbr = [float(v.mean()) for v in vr]
    # 0/1 shift matrices require one shared constant per direction
    for vs in (vbf, vbr):
        if abs(vs[0] - vs[1]) > 1e-6 * abs(vs[0]) or \
           abs(vs[0] - vs[2]) > 1e-6 * abs(vs[0]):
            ok = False

    c1f, c1r = 0.3 * df, 0.3 * dr
    af2 = (0.5 + 0.5 * np.cos(np.abs(angles).mean(axis=1))) \
        .astype(np.float32).reshape(NT, NP)
    gf = (0.7 * df * vbf[0]) * af2            # [128, 64]
    gr = (0.7 * dr * vbr[0]) * af2

    invt = (-np.arange(NT)) % NT
    invp = (-np.arange(NP)) % NP
    grm = gr[invt][:, invp]                   # mirrored rev gain field

    colphi = (np.arange(W) - HALO) % NP       # col -> phi
    ghat = np.empty((NT, 2, BPC, W), np.float32)
    ghat[:, 0] = (gf / c1f)[:, None, colphi]
    ghat[:, 1] = (grm / c1r)[:, None, colphi]

    W0f, wtf = _acc_weights(wf, c1f)
    W0r, wtr = _acc_weights(wr, c1r)

    # per-core packs (bf16): x0 (fwd plain / rev point-mirrored) and the
    # host-computed first gated update m~_0 = ghat (.) Op(x0)
    e3 = entry.reshape(B, NT, NP)
    em = e3[:, invt][:, :, invp]
    xm_list = []
    m0_list = []
    for c in range(NCORES):
        x0 = np.empty((NT, 2, BPC, W), np.float32)
        x0[:, 0] = e3[c * BPC:(c + 1) * BPC][:, :, colphi].transpose(1, 0, 2)
        x0[:, 1] = em[c * BPC:(c + 1) * BPC][:, :, colphi].transpose(1, 0, 2)
        x0 = x0.astype(ml_dtypes.bfloat16).astype(np.float32)
        x0m = np.roll(x0, 1, axis=0)          # theta-1 (S wraps partitions)
        p0 = np.zeros_like(x0)
        p0[..., 1:] = x0m[..., 1:] + x0[..., :-1] + x0m[..., :-1]
        m0 = (ghat.transpose(0, 1, 2, 3) * p0).astype(ml_dtypes.bfloat16)
        m0[..., 0] = 0
        xm = np.stack([x0.astype(ml_dtypes.bfloat16), m0], axis=1)
        xm_list.append(np.ascontiguousarray(xm))  # [NT, 2, 2, BPC, W]
        m0_list.append(m0.astype(np.float32))
    meta = dict(
        ok=ok, sig=sig,
        W0s=(W0f, W0r), wts=(tuple(wtf), tuple(wtr)),
        gs=np.ascontiguousarray(ghat), xm_list=xm_list, m0_list=m0_list,
        invt=invt, invp=invp, e3=e3, em=em,
    )
    return meta


_PROGRAM_CACHE = {}
LAST_RESULT = None


def kernel(**inputs):
    meta = _host_prep(inputs)
    if not meta["ok"]:
        return _reference_fallback(
            np.asarray(inputs["entry_probs"], np.float32),
            np.asarray(inputs["forward_adj"], np.float32),
            np.asarray(inputs["reverse_adj"], np.float32),
            inputs["forward_step_weights"], inputs["forward_decay"],
            inputs["reverse_step_weights"], inputs["reverse_decay"],
            inputs["interaction_weight"], np.asarray(inputs["bounce_angles"], np.float32))

    # If tracing is requested via BASS_TRACE but the image's antenv lacks
    # axon_hooks, provide the hook so run_bass_kernel_spmd doesn't crash.
    import os as _os
    if _os.environ.get("BASS_TRACE"):
        try:
            import antenv.axon_hooks  # noqa: F401
        except ImportError:
            try:
                import sys as _sys
                import types as _types
                import trn_agent_boot.trn_boot as _tb
                _hook = _tb._ntff_profile_via_ctypes("/opt/axon/libaxon_pjrt.so")
                _mod = _types.ModuleType("antenv.axon_hooks")
                _mod.get_axon_ntff_profile_hook = lambda: _hook
                _mod.set_axon_ntff_profile_hook = lambda h: None
                _sys.modules["antenv.axon_hooks"] = _mod
            except Exception:
                _os.environ.pop("BASS_TRACE", None)

    from concourse import bass_utils

    key = meta["wts"]
    if key not in _PROGRAM_CACHE:
        _PROGRAM_CACHE[key] = _build_program(meta["wts"])
    nc = _PROGRAM_CACHE[key]

    in_maps = [{"xm": meta["xm_list"][c], "gs": meta["gs"]}
               for c in range(NCORES)]
    res = bass_utils.run_bass_kernel_spmd(nc, in_maps, core_ids=list(range(NCORES)))
    global LAST_RESULT
    LAST_RESULT = res

    (W0f, W0r), (wtf, wtr) = meta["W0s"], meta["wts"]

    def gather(name, dtype):
        # [C, NT, 2, BPC, NP] -> [2, B, N]
        a = np.stack([np.asarray(r[name]).astype(dtype) for r in res.results])
        return a.transpose(2, 0, 3, 1, 4).reshape(2, B, N)

    acc8 = gather("acc8", np.float32)
    m9 = gather("m9", np.float32)
    m0 = np.stack([m[:, :, :, HALO:W] for m in meta["m0_list"]]) \
        .transpose(2, 0, 3, 1, 4).reshape(2, B, N)

    f = (W0f * meta["e3"].reshape(B, N) + wtf[0] * m0[0] + acc8[0]
         + wtf[S - 1] * m9[0])
    rm = (W0r * meta["em"].reshape(B, N) + wtr[0] * m0[1] + acc8[1]
          + wtr[S - 1] * m9[1])
    rm3 = rm.reshape(B, NT, NP)
    r = rm3[:, meta["invt"]][:, :, meta["invp"]].reshape(B, N)
    f = f.astype(np.float32)
    r = r.astype(np.float32)
    inter = (f * r).astype(np.float32)
    comb = (f + r + np.float32(meta["sig"]) * inter).astype(np.float32)
    return comb, inter
